# revision 19
# baseline (speedup 1.0000x reference)
"""DeepseekV4 MLA attention on 8 trn2 cores.

Sharding: tensor-parallel over heads (8 heads = 1 output group per core).
Stage A (q-lora down-proj + kv latent) is sharded over sequence blocks and
all-gathered; attention + group output projection run head-parallel with the
tiny kv latent replicated; the final dense projection is sharded over output
channels using an all-gather of the per-group activations.

kernel(**inputs) takes the full unsharded inputs and returns the full output.
"""
import numpy as np
import ml_dtypes
from contextlib import ExitStack

import concourse.bass as bass
import concourse.tile as tile
import concourse.mybir as mybir
from concourse import bacc
from concourse.bass_utils import run_bass_kernel_spmd
from concourse.masks import make_identity

BF = ml_dtypes.bfloat16
F32 = mybir.dt.float32
BF16 = mybir.dt.bfloat16
AF = mybir.ActivationFunctionType
AX = mybir.AxisListType

S, HID = 1024, 7168
H, D, ROPE, NOPE = 64, 512, 64, 448
G, R, HPG = 8, 1024, 8
QL = 1536
EPS = 1e-6
SCALE = D ** -0.5

NC = 8           # cores
SBLK = 128       # seq rows per core in stage A
HPC = 8          # heads per core
OUTC = HID // NC  # 896 output channels per core
KC = HID // 128   # 56 contraction chunks over hidden
QC = QL // 128    # 12 contraction chunks over q-lora dim

TN1 = QL * SBLK       # tnT bytes-elements in gather-1 payload
TK1 = D * SBLK        # kvT part
TKV = SBLK * D        # kv natural part
CC1N = TN1 + TK1 + TKV
CC2N = R * 512        # out_gT half payload
CORE_IDS = list(range(NC))

_CACHE = {}


def _build():
    nc = bacc.Bacc("TRN2", target_bir_lowering=False, debug=False, num_devices=NC)

    # ---- per-core external inputs ----
    xT_d = nc.dram_tensor("xT", [HID, SBLK], BF16, kind="ExternalInput").ap()
    wA_d = nc.dram_tensor("wA", [HID, 2048], BF16, kind="ExternalInput").ap()
    wqbT_d = nc.dram_tensor("wqbT", [QL, HPC * D], BF16, kind="ExternalInput").ap()
    woaT_d = nc.dram_tensor("woaT", [HPG * D, R], BF16, kind="ExternalInput").ap()
    wobT_d = nc.dram_tensor("wobT", [G * R, OUTC], BF16, kind="ExternalInput").ap()
    kvwb_d = nc.dram_tensor("kvwb", [128, D], F32, kind="ExternalInput").ap()
    csA_d = nc.dram_tensor("csA", [SBLK, ROPE], F32, kind="ExternalInput").ap()
    csF_d = nc.dram_tensor("csF", [S, ROPE], F32, kind="ExternalInput").ap()
    sinkb_d = nc.dram_tensor("sinkb", [128, HPC], F32, kind="ExternalInput").ap()
    cmask_d = nc.dram_tensor("cmask", [128, 128], F32, kind="ExternalInput").ap()
    outY_d = nc.dram_tensor("outY", [S, OUTC], F32, kind="ExternalOutput").ap()

    # ---- internal dram for collectives ----
    cc1i = nc.dram_tensor("cc1i", [CC1N], BF16).ap()
    cc1o = nc.dram_tensor("cc1o", [NC * CC1N], BF16, addr_space="Shared").ap()
    cc2i = [nc.dram_tensor(f"cc2i{h}", [CC2N], BF16).ap() for h in range(2)]
    cc2o = [nc.dram_tensor(f"cc2o{h}", [NC * CC2N], BF16, addr_space="Shared").ap()
            for h in range(2)]

    with tile.TileContext(nc) as tc, ExitStack() as octx:
        consts = octx.enter_context(tc.tile_pool(name="consts", bufs=1))
        ident = consts.tile([128, 128], BF16)
        make_identity(nc, ident)
        cmask_sb = consts.tile([128, 128], F32)
        nc.sync.dma_start(out=cmask_sb, in_=cmask_d)
        kvwb = consts.tile([128, D], F32)
        nc.sync.dma_start(out=kvwb, in_=kvwb_d)
        csA_sb = consts.tile([128, ROPE], F32)
        nc.sync.dma_start(out=csA_sb, in_=csA_d)
        sinkb_sb = consts.tile([128, HPC], F32)
        nc.sync.dma_start(out=sinkb_sb, in_=sinkb_d)
        epsv = consts.tile([128, 1], F32)
        nc.vector.memset(epsv, EPS)

        # ================= PHASE A: sharded q-lora + kv latent =============
        with tc.tile_pool(name="xtp", bufs=1) as xtp, \
             tc.tile_pool(name="wap", bufs=4) as wap, \
             tc.tile_pool(name="psA", bufs=1, space="PSUM") as psA, \
             tc.tile_pool(name="tpsA", bufs=2, space="PSUM") as tpsA, \
             tc.tile_pool(name="sbA", bufs=2) as sbA:
            xt = xtp.tile([128, KC, SBLK], BF16)
            nc.sync.dma_start(out=xt, in_=xT_d.rearrange("(c k) s -> k c s", k=128))
            ps = psA.tile([128, 2048], F32)
            for k in range(KC):
                wa_t = wap.tile([128, 2048], BF16, tag="wa")
                nc.sync.dma_start(
                    out=wa_t,
                    in_=wA_d.rearrange("(c k) n -> k c n", k=128)[:, k, :])
                for n in range(4):
                    nc.tensor.matmul(ps[:, n * 512:(n + 1) * 512], xt[:, k, :],
                                     wa_t[:, n * 512:(n + 1) * 512],
                                     start=(k == 0), stop=(k == KC - 1))
            # rms-norm of tn (q-lora, weight folded into wqbT on host)
            scr = sbA.tile([128, QL], BF16, tag="scrA")
            ssq = sbA.tile([128, 1], F32, tag="ssqA")
            nc.scalar.activation(out=scr, in_=ps[:, 0:QL], func=AF.Square,
                                 accum_out=ssq)
            sd = sbA.tile([128, 1], F32, tag="sdA")
            nc.scalar.activation(out=sd, in_=ssq, func=AF.Sqrt, bias=epsv,
                                 scale=1.0 / QL)
            r1 = sbA.tile([128, 1], F32, tag="r1A")
            nc.vector.reciprocal(r1, sd)
            tn = sbA.tile([128, QL], BF16, tag="tn")
            nc.vector.tensor_scalar_mul(tn, ps[:, 0:QL], r1)
            # rms-norm of kv latent + weight + rope
            scr2 = sbA.tile([128, D], BF16, tag="scr2A")
            ssk = sbA.tile([128, 1], F32, tag="sskA")
            nc.scalar.activation(out=scr2, in_=ps[:, QL:QL + D], func=AF.Square,
                                 accum_out=ssk)
            sdk = sbA.tile([128, 1], F32, tag="sdkA")
            nc.scalar.activation(out=sdk, in_=ssk, func=AF.Sqrt, bias=epsv,
                                 scale=1.0 / D)
            rk = sbA.tile([128, 1], F32, tag="rkA")
            nc.vector.reciprocal(rk, sdk)
            kvf = sbA.tile([128, D], F32, tag="kvfA")
            nc.vector.tensor_scalar_mul(kvf, ps[:, QL:QL + D], rk)
            kvn = sbA.tile([128, D], F32, tag="kvnA")
            nc.vector.tensor_mul(kvn, kvf, kvwb)
            t1 = sbA.tile([128, 32], F32, tag="t1A")
            t2 = sbA.tile([128, 32], F32, tag="t2A")
            t3 = sbA.tile([128, 32], F32, tag="t3A")
            t4 = sbA.tile([128, 32], F32, tag="t4A")
            nc.vector.tensor_mul(t1, kvn[:, 448:480], csA_sb[:, 0:32])
            nc.vector.tensor_mul(t2, kvn[:, 480:512], csA_sb[:, 32:64])
            nc.vector.tensor_mul(t3, kvn[:, 448:480], csA_sb[:, 32:64])
            nc.vector.tensor_mul(t4, kvn[:, 480:512], csA_sb[:, 0:32])
            kvb = sbA.tile([128, D], BF16, tag="kvbA")
            nc.vector.tensor_copy(kvb[:, 0:448], kvn[:, 0:448])
            nc.vector.tensor_sub(kvb[:, 448:480], t1, t2)
            nc.vector.tensor_add(kvb[:, 480:512], t3, t4)
            # transposes of tn and kv for the gathered layouts
            tnT = sbA.tile([128, QC, 128], BF16, tag="tnT")
            for c in range(QC):
                tp = tpsA.tile([128, 128], BF16, tag="tpA")
                nc.tensor.transpose(tp, tn[:, c * 128:(c + 1) * 128], ident)
                nc.vector.tensor_copy(tnT[:, c, :], tp)
            kvT = sbA.tile([128, 4, 128], BF16, tag="kvT")
            for c in range(4):
                tp = tpsA.tile([128, 128], BF16, tag="tpA")
                nc.tensor.transpose(tp, kvb[:, c * 128:(c + 1) * 128], ident)
                nc.vector.tensor_copy(kvT[:, c, :], tp)
            nc.sync.dma_start(
                out=cc1i[0:TN1].rearrange("(c p s) -> p c s", p=128, s=SBLK),
                in_=tnT)
            nc.sync.dma_start(
                out=cc1i[TN1:TN1 + TK1].rearrange("(c p s) -> p c s", p=128, s=SBLK),
                in_=kvT)
            nc.sync.dma_start(
                out=cc1i[TN1 + TK1:CC1N].rearrange("(p d) -> p d", p=128),
                in_=kvb)
            nc.gpsimd.collective_compute(
                "AllGather", mybir.AluOpType.bypass, replica_groups=[CORE_IDS],
                ins=[cc1i[:]], outs=[cc1o[:]])

        cc1v = cc1o.rearrange("(g n) -> g n", g=NC)

        # ============ PHASE B: q up-projection + per-head norm/rope ========
        qallp = octx.enter_context(tc.tile_pool(name="qallp", bufs=1))
        qall = qallp.tile([128, 8, HPC * D], BF16)

        with tc.tile_pool(name="BT", bufs=1) as BT, \
             tc.tile_pool(name="wqbp", bufs=1) as wqbp, \
             tc.tile_pool(name="psB", bufs=4, space="PSUM") as psB, \
             tc.tile_pool(name="sbB", bufs=4) as sbB:
            tnT_f = BT.tile([128, QC, S], BF16)
            for c in range(QC):
                nc.sync.dma_start(
                    out=tnT_f[:, c, :].rearrange("p (g s) -> p g s", g=NC),
                    in_=cc1v[:, c * 16384:(c + 1) * 16384]
                        .rearrange("g (p s) -> p g s", p=128))
            csF_sb = BT.tile([128, 8, ROPE], F32)
            nc.sync.dma_start(out=csF_sb,
                              in_=csF_d.rearrange("(t p) c -> p t c", p=128))
            for wh in range(2):
              wqb = wqbp.tile([128, QC, 2048], BF16, tag="wqbh",
                              name=f"wqb_{wh}")
              nc.sync.dma_start(
                  out=wqb,
                  in_=wqbT_d.rearrange("(c k) n -> k c n", k=128)
                      [:, :, wh * 2048:(wh + 1) * 2048])
              for st in range(8):
                qpt = [psB.tile([128, 1024], F32, tag="qps",
                                name=f"qps_{wh}_{st}_{i}") for i in range(2)]
                for c in range(QC):
                    lh = tnT_f[:, c, st * 128:(st + 1) * 128]
                    for hl in range(4):
                        nc.tensor.matmul(
                            qpt[hl // 2][:, (hl % 2) * 512:(hl % 2 + 1) * 512],
                            lh, wqb[:, c, hl * 512:(hl + 1) * 512],
                            start=(c == 0), stop=(c == QC - 1))
                for hl in range(4):
                    if True:
                        h = wh * 4 + hl
                        qsl = qpt[hl // 2][:, (hl % 2) * 512:(hl % 2 + 1) * 512]
                        scr = sbB.tile([128, D], BF16, tag="scrB")
                        ssq = sbB.tile([128, 1], F32, tag="ssqB")
                        nc.scalar.activation(out=scr, in_=qsl, func=AF.Square,
                                             accum_out=ssq)
                        sd = sbB.tile([128, 1], F32, tag="sdB")
                        nc.scalar.activation(out=sd, in_=ssq, func=AF.Sqrt,
                                             bias=epsv, scale=1.0 / D)
                        rq = sbB.tile([128, 1], F32, tag="rqB")
                        nc.vector.reciprocal(rq, sd)
                        r2 = sbB.tile([128, 1], F32, tag="r2B")
                        nc.vector.tensor_scalar_mul(r2, rq, SCALE)
                        qf = sbB.tile([128, D], F32, tag="qfB")
                        nc.vector.tensor_scalar_mul(qf, qsl, r2)
                        cs = csF_sb[:, st, :]
                        t1 = sbB.tile([128, 32], F32, tag="t1B")
                        t2 = sbB.tile([128, 32], F32, tag="t2B")
                        t3 = sbB.tile([128, 32], F32, tag="t3B")
                        t4 = sbB.tile([128, 32], F32, tag="t4B")
                        nc.vector.tensor_mul(t1, qf[:, 448:480], cs[:, 0:32])
                        nc.vector.tensor_mul(t2, qf[:, 480:512], cs[:, 32:64])
                        nc.vector.tensor_mul(t3, qf[:, 448:480], cs[:, 32:64])
                        nc.vector.tensor_mul(t4, qf[:, 480:512], cs[:, 0:32])
                        base = h * 512
                        nc.vector.tensor_copy(qall[:, st, base:base + 448],
                                              qf[:, 0:448])
                        nc.vector.tensor_sub(qall[:, st, base + 448:base + 480],
                                             t1, t2)
                        nc.vector.tensor_add(qall[:, st, base + 480:base + 512],
                                             t3, t4)

        # ============ PHASE C/D: attention + group output projection =======
        with tc.tile_pool(name="kvp", bufs=1) as kvp, \
             tc.tile_pool(name="woap", bufs=2) as woap, \
             tc.tile_pool(name="oTp", bufs=1) as oTp, \
             tc.tile_pool(name="psS", bufs=2, space="PSUM") as psS, \
             tc.tile_pool(name="psO", bufs=2, space="PSUM") as psO, \
             tc.tile_pool(name="psT", bufs=2, space="PSUM") as psT, \
             tc.tile_pool(name="psD", bufs=2, space="PSUM") as psD, \
             tc.tile_pool(name="sbC", bufs=6) as sbC, \
             tc.tile_pool(name="pP", bufs=4) as pP, \
             tc.tile_pool(name="pTp", bufs=4) as pTp, \
             tc.tile_pool(name="sbD", bufs=4) as sbD:
            kvT_f = kvp.tile([128, 4, S], BF16)
            for c in range(4):
                nc.sync.dma_start(
                    out=kvT_f[:, c, :].rearrange("p (g s) -> p g s", g=NC),
                    in_=cc1v[:, TN1 + c * 16384:TN1 + (c + 1) * 16384]
                        .rearrange("g (p s) -> p g s", p=128))
            kv_f = kvp.tile([128, NC, D], BF16)
            for j in range(NC):
                nc.sync.dma_start(
                    out=kv_f[:, j, :],
                    in_=cc1v[j, TN1 + TK1:CC1N].rearrange("(p d) -> p d", p=128))
            oT_all = oTp.tile([128, 32, 4, 128], BF16)
            for st in range(8):
                for h in range(HPC):
                    qt_ps = psT.tile([128, 4, 128], BF16, tag="tps")
                    for dc in range(4):
                        nc.tensor.transpose(
                            qt_ps[:, dc, :],
                            qall[:, st, h * 512 + dc * 128:h * 512 + (dc + 1) * 128],
                            ident)
                    qt = sbC.tile([128, 4, 128], BF16, tag="qt")
                    nc.vector.tensor_copy(qt, qt_ps)
                    ncols = (st + 1) * 128
                    chunks = [(0, min(512, ncols))]
                    if ncols > 512:
                        chunks.append((512, ncols - 512))
                    scps, mxs = [], []
                    for (off, w) in chunks:
                        sp = psS.tile([128, 512], F32, tag="sc")
                        for dc in range(4):
                            nc.tensor.matmul(sp[:, :w], qt[:, dc, :],
                                             kvT_f[:, dc, off:off + w],
                                             start=(dc == 0), stop=(dc == 3))
                        if off + w == ncols:
                            nc.vector.tensor_add(sp[:, w - 128:w],
                                                 sp[:, w - 128:w], cmask_sb)
                        mx = sbC.tile([128, 1], F32, tag="mx")
                        nc.vector.reduce_max(out=mx, in_=sp[:, :w], axis=AX.X)
                        scps.append((sp, off, w))
                        mxs.append(mx)
                    mm = sbC.tile([128, 1], F32, tag="mm")
                    if len(mxs) == 2:
                        nc.vector.tensor_max(mm, mxs[0], mxs[1])
                    else:
                        nc.vector.tensor_copy(mm, mxs[0])
                    m2 = sbC.tile([128, 1], F32, tag="m2")
                    nc.vector.tensor_max(m2, mm, sinkb_sb[:, h:h + 1])
                    negm = sbC.tile([128, 1], F32, tag="negm")
                    nc.vector.tensor_scalar_mul(negm, m2, -1.0)
                    psb = pP.tile([128, 1024], BF16, tag="p")
                    sums = []
                    for (sp, off, w) in scps:
                        sm = sbC.tile([128, 1], F32, tag="sm")
                        nc.scalar.activation(out=psb[:, off:off + w],
                                             in_=sp[:, :w], func=AF.Exp,
                                             bias=negm, scale=1.0, accum_out=sm)
                        sums.append(sm)
                    se = sbC.tile([128, 1], F32, tag="se")
                    nc.scalar.activation(out=se, in_=sinkb_sb[:, h:h + 1],
                                         func=AF.Exp, bias=negm, scale=1.0)
                    dn = sbC.tile([128, 1], F32, tag="dn")
                    nc.vector.tensor_add(dn, sums[0], se)
                    if len(sums) == 2:
                        dn2 = sbC.tile([128, 1], F32, tag="dn2")
                        nc.vector.tensor_add(dn2, dn, sums[1])
                        dn = dn2
                    rd = sbC.tile([128, 1], F32, tag="rd")
                    nc.vector.reciprocal(rd, dn)
                    pT = pTp.tile([128, 8, 128], BF16, tag="pT")
                    for j in range(st + 1):
                        nc.sync.dma_start(out=pT[:, j, :],
                                          in_=psb[:, j * 128:(j + 1) * 128],
                                          transpose=True)
                    ops = psO.tile([128, 512], F32, tag="o")
                    for j in range(st + 1):
                        nc.tensor.matmul(ops, pT[:, j, :], kv_f[:, j, :],
                                         start=(j == 0), stop=(j == st))
                    obf = sbC.tile([128, D], BF16, tag="obf")
                    nc.vector.tensor_scalar_mul(obf, ops, rd)
                    ot_ps = psT.tile([128, 4, 128], BF16, tag="tps")
                    for dc in range(4):
                        nc.tensor.transpose(ot_ps[:, dc, :],
                                            obf[:, dc * 128:(dc + 1) * 128],
                                            ident)
                    nc.vector.tensor_copy(oT_all[:, h * 4:(h + 1) * 4, st % 4, :],
                                          ot_ps)
                if st in (3, 7):
                    half = st // 4
                    for rc in range(8):
                        woa_rc = woap.tile([128, 32, 128], BF16, tag="woa",
                                           name=f"woa_{half}_{rc}")
                        nc.sync.dma_start(
                            out=woa_rc,
                            in_=woaT_d.rearrange("(c k) n -> k c n", k=128)
                                [:, :, rc * 128:(rc + 1) * 128])
                        dps = psD.tile([128, 512], F32, tag="dps")
                        for dc in range(32):
                            nc.tensor.matmul(
                                dps, woa_rc[:, dc, :],
                                oT_all[:, dc, :, :].rearrange("p a b -> p (a b)"),
                                start=(dc == 0), stop=(dc == 31))
                        ob = sbD.tile([128, 512], BF16, tag="ob")
                        nc.vector.tensor_copy(ob, dps)
                        nc.sync.dma_start(
                            out=cc2i[half][rc * 65536:(rc + 1) * 65536]
                                .rearrange("(p s) -> p s", p=128),
                            in_=ob)
                    nc.gpsimd.collective_compute(
                        "AllGather", mybir.AluOpType.bypass,
                        replica_groups=[CORE_IDS],
                        ins=[cc2i[half][:]], outs=[cc2o[half][:]])

        # ================= PHASE E: final dense projection =================
        with tc.tile_pool(name="wobp", bufs=1) as wobp, \
             tc.tile_pool(name="psE", bufs=2, space="PSUM") as psE, \
             tc.tile_pool(name="ctp", bufs=4) as ctp, \
             tc.tile_pool(name="sbE", bufs=3) as sbE:
            wob = wobp.tile([128, 64, OUTC], BF16)
            nc.sync.dma_start(out=wob,
                              in_=wobT_d.rearrange("(c k) n -> k c n", k=128))
            for sb2 in range(2):
                cc2v = cc2o[sb2].rearrange("(g n) -> g n", g=NC)
                for sp2 in range(2):
                    eps_t = [psE.tile([128, OUTC], F32, tag="eps",
                                      name=f"eps_{sb2}_{sp2}_{i}")
                             for i in range(2)]
                    for rc in range(64):
                        gp, j = rc // 8, rc % 8
                        ct = ctp.tile([128, 512], BF16, tag="ct")
                        nc.sync.dma_start(
                            out=ct,
                            in_=cc2v[gp, j * 65536:(j + 1) * 65536]
                                .rearrange("(p s) -> p s", p=128))
                        for e2 in range(2):
                            stl = sp2 * 2 + e2
                            lh = ct[:, stl * 128:(stl + 1) * 128]
                            nc.tensor.matmul(eps_t[e2][:, 0:512], lh,
                                             wob[:, rc, 0:512],
                                             start=(rc == 0), stop=(rc == 63))
                            nc.tensor.matmul(eps_t[e2][:, 512:OUTC], lh,
                                             wob[:, rc, 512:OUTC],
                                             start=(rc == 0), stop=(rc == 63))
                    for e2 in range(2):
                        stile = sb2 * 4 + sp2 * 2 + e2
                        of = sbE.tile([128, OUTC], F32, tag="of")
                        nc.vector.tensor_copy(of, eps_t[e2])
                        nc.sync.dma_start(
                            out=outY_d[stile * 128:(stile + 1) * 128, :],
                            in_=of)

    nc.compile()
    return nc


def _host_prep(x, freqs_cis, wq_a, q_norm_w, wq_b, wkv, kv_norm_w,
               wo_a_w, wo_b, attn_sink):
    perm = np.concatenate([np.arange(NOPE),
                           NOPE + 2 * np.arange(ROPE // 2),
                           NOPE + 1 + 2 * np.arange(ROPE // 2)])
    x2 = np.asarray(x, np.float32).reshape(S, HID)
    wqa_T = np.asarray(wq_a, np.float32).T                      # [HID, QL]
    wkv_p = np.asarray(wkv, np.float32)[perm, :]                # [D, HID]
    wA = np.ascontiguousarray(
        np.concatenate([wqa_T, wkv_p.T], axis=1)).astype(BF)    # [HID, 2048]
    wqb_eff = np.asarray(wq_b, np.float32) * np.asarray(q_norm_w, np.float32)[None, :]
    wqb_r = wqb_eff.reshape(H, D, QL)[:, perm, :]               # [H, D, QL]
    fc = np.asarray(freqs_cis, np.float32)
    csF = np.ascontiguousarray(
        np.concatenate([fc[:, :, 0], fc[:, :, 1]], axis=1))     # [S, 64]
    kvw = np.asarray(kv_norm_w, np.float32)[perm]
    kvwb = np.ascontiguousarray(np.tile(kvw[None, :], (128, 1)))
    woa = np.asarray(wo_a_w, np.float32).reshape(G, R, HPG, D)[:, :, :, perm] \
        .reshape(G, R, HPG * D)
    wob = np.asarray(wo_b, np.float32)
    ii = np.arange(128)
    cmask = np.where(ii[None, :] <= ii[:, None], 0.0, -1e30).astype(np.float32)
    sink = np.asarray(attn_sink, np.float32)

    in_maps = []
    for g in range(NC):
        xT = np.ascontiguousarray(x2[g * SBLK:(g + 1) * SBLK, :].T).astype(BF)
        wqbT = np.ascontiguousarray(
            wqb_r[g * HPC:(g + 1) * HPC].reshape(HPC * D, QL).T).astype(BF)
        woaT = np.ascontiguousarray(woa[g].T).astype(BF)
        wobT = np.ascontiguousarray(
            wob[g * OUTC:(g + 1) * OUTC, :].T).astype(BF)
        sinkb = np.ascontiguousarray(
            np.tile(sink[g * HPC:(g + 1) * HPC][None, :], (128, 1)))
        csA = np.ascontiguousarray(csF[g * SBLK:(g + 1) * SBLK])
        in_maps.append({
            "xT": xT, "wA": wA, "wqbT": wqbT, "woaT": woaT, "wobT": wobT,
            "kvwb": kvwb, "csA": csA, "csF": csF, "sinkb": sinkb,
            "cmask": cmask,
        })
    return in_maps


def _make_runner(nc, chain=1, donate=True):
    """Build the jitted 8-core PJRT executor once (mirrors the multi-core
    branch of bass2jax.run_bass_via_pjrt, but caches the jitted callable).

    chain>1 executes the NEFF `chain` times back-to-back with a data
    dependency through the donated output buffer — used to measure device
    execution time as a slope, independent of host/tunnel transfer costs."""
    import jax
    from jax.experimental.shard_map import shard_map
    from jax.sharding import Mesh, PartitionSpec
    from concourse import bass2jax

    bass2jax.install_neuronx_cc_hook()
    partition_name = (nc.partition_id_tensor.name
                      if nc.partition_id_tensor else None)
    in_names, out_names, out_avals = [], [], []
    for alloc in nc.m.functions[0].allocations:
        if not isinstance(alloc, mybir.MemoryLocationSet):
            continue
        name = alloc.memorylocations[0].name
        if alloc.kind == "ExternalInput":
            if name != partition_name:
                in_names.append(name)
        elif alloc.kind == "ExternalOutput":
            out_names.append(name)
            out_avals.append(jax.core.ShapedArray(
                tuple(alloc.tensor_shape), mybir.dt.np(alloc.dtype)))
    n_params = len(in_names)
    all_names = list(in_names) + list(out_names)
    if partition_name is not None:
        all_names.append(partition_name)
    all_names = tuple(all_names)
    donate_idx = (tuple(range(n_params, n_params + len(out_names)))
                  if donate else ())

    def _body(*args):
        ins = list(args[:n_params])
        outs = list(args[n_params:])
        for _ in range(chain):
            operands = ins + outs
            if partition_name is not None:
                operands.append(bass2jax.partition_id_tensor())
            outs = list(bass2jax._bass_exec_p.bind(
                *operands, out_avals=tuple(out_avals), in_names=all_names,
                out_names=tuple(out_names), lowering_input_output_aliases=(),
                sim_require_finite=True, sim_require_nnan=True, nc=nc))
        return tuple(outs)

    devices = jax.devices()[:NC]
    mesh = Mesh(np.asarray(devices), ("core",))
    in_specs = (PartitionSpec("core"),) * (n_params + len(out_names))
    out_specs = (PartitionSpec("core"),) * len(out_names)
    sharded = jax.jit(
        shard_map(_body, mesh=mesh, in_specs=in_specs, out_specs=out_specs,
                  check_rep=False),
        donate_argnums=donate_idx, keep_unused=True)
    return {"sharded": sharded, "in_names": in_names, "out_names": out_names,
            "out_avals": out_avals, "mesh": mesh}


def get_runner(chain=1, donate=True):
    key = f"runner_{chain}_{donate}"
    if key not in _CACHE:
        if "nc" not in _CACHE:
            _CACHE["nc"] = _build()
        _CACHE[key] = _make_runner(_CACHE["nc"], chain=chain, donate=donate)
    return _CACHE[key]


def concat_inputs(in_maps, runner):
    return [np.concatenate([in_maps[c][n] for c in range(NC)], axis=0)
            for n in runner["in_names"]]


def make_zero_outs(runner):
    return [np.zeros((NC * av.shape[0], *av.shape[1:]), av.dtype)
            for av in runner["out_avals"]]


def kernel(x, freqs_cis, wq_a, q_norm_w, wq_b, wkv, kv_norm_w,
           wo_a_w, wo_b, attn_sink):
    runner = get_runner(donate=False)
    in_maps = _host_prep(x, freqs_cis, wq_a, q_norm_w, wq_b, wkv, kv_norm_w,
                         wo_a_w, wo_b, attn_sink)
    out_arrs = runner["sharded"](*concat_inputs(in_maps, runner),
                                 *make_zero_outs(runner))
    idx = runner["out_names"].index("outY")
    outY = np.asarray(out_arrs[idx]).reshape(NC, S, OUTC)
    out = np.empty((1, S, HID), np.float32)
    for g in range(NC):
        out[0, :, g * OUTC:(g + 1) * OUTC] = outY[g]
    return out


# revision 20
# speedup vs baseline: 2.3465x; 2.3465x over previous
"""DeepseekV4 MLA attention on 8 trn2 cores.

Sharding: tensor-parallel over heads (8 heads = 1 output group per core).
Stage A (q-lora down-proj + kv latent) is sharded over sequence blocks and
all-gathered; attention + group output projection run head-parallel with the
tiny kv latent replicated; the final dense projection is sharded over output
channels using an all-gather of the per-group activations.

kernel(**inputs) takes the full unsharded inputs and returns the full output.
"""
import numpy as np
import ml_dtypes
from contextlib import ExitStack

import concourse.bass as bass
import concourse.tile as tile
import concourse.mybir as mybir
from concourse import bacc
from concourse.bass_utils import run_bass_kernel_spmd
from concourse.masks import make_identity

BF = ml_dtypes.bfloat16
F32 = mybir.dt.float32
BF16 = mybir.dt.bfloat16
AF = mybir.ActivationFunctionType
AX = mybir.AxisListType

S, HID = 1024, 7168
H, D, ROPE, NOPE = 64, 512, 64, 448
G, R, HPG = 8, 1024, 8
QL = 1536
EPS = 1e-6
SCALE = D ** -0.5

NC = 8           # cores
SBLK = 128       # seq rows per core in stage A
HPC = 8          # heads per core
OUTC = HID // NC  # 896 output channels per core
KC = HID // 128   # 56 contraction chunks over hidden
QC = QL // 128    # 12 contraction chunks over q-lora dim

TN1 = QL * SBLK       # tnT bytes-elements in gather-1 payload
TK1 = D * SBLK        # kvT part
TKV = SBLK * D        # kv natural part
CC1N = TN1 + TK1 + TKV
CC2N = R * 512        # out_gT half payload
CORE_IDS = list(range(NC))

_CACHE = {}


def _build():
    nc = bacc.Bacc("TRN2", target_bir_lowering=False, debug=False, num_devices=NC)

    # ---- per-core external inputs ----
    xT_d = nc.dram_tensor("xT", [HID, SBLK], BF16, kind="ExternalInput").ap()
    wA_d = nc.dram_tensor("wA", [HID, 2048], BF16, kind="ExternalInput").ap()
    wqbT_d = nc.dram_tensor("wqbT", [QL, HPC * D], BF16, kind="ExternalInput").ap()
    woaT_d = nc.dram_tensor("woaT", [HPG * D, R], BF16, kind="ExternalInput").ap()
    wobT_d = nc.dram_tensor("wobT", [G * R, OUTC], BF16, kind="ExternalInput").ap()
    kvwb_d = nc.dram_tensor("kvwb", [128, D], F32, kind="ExternalInput").ap()
    csA_d = nc.dram_tensor("csA", [SBLK, ROPE], F32, kind="ExternalInput").ap()
    csF_d = nc.dram_tensor("csF", [S, ROPE], F32, kind="ExternalInput").ap()
    sinkb_d = nc.dram_tensor("sinkb", [128, HPC], F32, kind="ExternalInput").ap()
    cmask_d = nc.dram_tensor("cmask", [128, 128], F32, kind="ExternalInput").ap()
    outY_d = nc.dram_tensor("outY", [S, OUTC], F32, kind="ExternalOutput").ap()

    # ---- internal dram for collectives ----
    cc1i = nc.dram_tensor("cc1i", [CC1N], BF16).ap()
    cc1o = nc.dram_tensor("cc1o", [NC * CC1N], BF16, addr_space="Shared").ap()
    cc2i = [nc.dram_tensor(f"cc2i{h}", [CC2N], BF16).ap() for h in range(2)]
    cc2o = [nc.dram_tensor(f"cc2o{h}", [NC * CC2N], BF16, addr_space="Shared").ap()
            for h in range(2)]

    with tile.TileContext(nc) as tc, ExitStack() as octx:
        consts = octx.enter_context(tc.tile_pool(name="consts", bufs=1))
        ident = consts.tile([128, 128], BF16)
        make_identity(nc, ident)
        cmask_sb = consts.tile([128, 128], F32)
        nc.sync.dma_start(out=cmask_sb, in_=cmask_d)
        kvwb = consts.tile([128, D], F32)
        nc.sync.dma_start(out=kvwb, in_=kvwb_d)
        csA_sb = consts.tile([128, ROPE], F32)
        nc.sync.dma_start(out=csA_sb, in_=csA_d)
        sinkb_sb = consts.tile([128, HPC], F32)
        nc.sync.dma_start(out=sinkb_sb, in_=sinkb_d)
        epsv = consts.tile([128, 1], F32)
        nc.vector.memset(epsv, EPS)

        # ================= PHASE A: sharded q-lora + kv latent =============
        with tc.tile_pool(name="xtp", bufs=1) as xtp, \
             tc.tile_pool(name="wap", bufs=4) as wap, \
             tc.tile_pool(name="psA", bufs=1, space="PSUM") as psA, \
             tc.tile_pool(name="tpsA", bufs=2, space="PSUM") as tpsA, \
             tc.tile_pool(name="sbA", bufs=2) as sbA:
            xt = xtp.tile([128, KC, SBLK], BF16)
            nc.sync.dma_start(out=xt, in_=xT_d.rearrange("(c k) s -> k c s", k=128))
            ps = psA.tile([128, 2048], F32)
            for k in range(KC):
                wa_t = wap.tile([128, 2048], BF16, tag="wa")
                nc.sync.dma_start(
                    out=wa_t,
                    in_=wA_d.rearrange("(c k) n -> k c n", k=128)[:, k, :])
                for n in range(4):
                    nc.tensor.matmul(ps[:, n * 512:(n + 1) * 512], xt[:, k, :],
                                     wa_t[:, n * 512:(n + 1) * 512],
                                     start=(k == 0), stop=(k == KC - 1))
            # rms-norm of tn (q-lora, weight folded into wqbT on host)
            scr = sbA.tile([128, QL], BF16, tag="scrA")
            ssq = sbA.tile([128, 1], F32, tag="ssqA")
            nc.scalar.activation(out=scr, in_=ps[:, 0:QL], func=AF.Square,
                                 accum_out=ssq)
            sd = sbA.tile([128, 1], F32, tag="sdA")
            nc.scalar.activation(out=sd, in_=ssq, func=AF.Sqrt, bias=epsv,
                                 scale=1.0 / QL)
            r1 = sbA.tile([128, 1], F32, tag="r1A")
            nc.vector.reciprocal(r1, sd)
            tn = sbA.tile([128, QL], BF16, tag="tn")
            nc.vector.tensor_scalar_mul(tn, ps[:, 0:QL], r1)
            # rms-norm of kv latent + weight + rope
            scr2 = sbA.tile([128, D], BF16, tag="scr2A")
            ssk = sbA.tile([128, 1], F32, tag="sskA")
            nc.scalar.activation(out=scr2, in_=ps[:, QL:QL + D], func=AF.Square,
                                 accum_out=ssk)
            sdk = sbA.tile([128, 1], F32, tag="sdkA")
            nc.scalar.activation(out=sdk, in_=ssk, func=AF.Sqrt, bias=epsv,
                                 scale=1.0 / D)
            rk = sbA.tile([128, 1], F32, tag="rkA")
            nc.vector.reciprocal(rk, sdk)
            kvf = sbA.tile([128, D], F32, tag="kvfA")
            nc.vector.tensor_scalar_mul(kvf, ps[:, QL:QL + D], rk)
            kvn = sbA.tile([128, D], F32, tag="kvnA")
            nc.vector.tensor_mul(kvn, kvf, kvwb)
            t1 = sbA.tile([128, 32], F32, tag="t1A")
            t2 = sbA.tile([128, 32], F32, tag="t2A")
            t3 = sbA.tile([128, 32], F32, tag="t3A")
            t4 = sbA.tile([128, 32], F32, tag="t4A")
            nc.vector.tensor_mul(t1, kvn[:, 448:480], csA_sb[:, 0:32])
            nc.vector.tensor_mul(t2, kvn[:, 480:512], csA_sb[:, 32:64])
            nc.vector.tensor_mul(t3, kvn[:, 448:480], csA_sb[:, 32:64])
            nc.vector.tensor_mul(t4, kvn[:, 480:512], csA_sb[:, 0:32])
            kvb = sbA.tile([128, D], BF16, tag="kvbA")
            nc.vector.tensor_copy(kvb[:, 0:448], kvn[:, 0:448])
            nc.vector.tensor_sub(kvb[:, 448:480], t1, t2)
            nc.vector.tensor_add(kvb[:, 480:512], t3, t4)
            # transposes of tn and kv for the gathered layouts
            tnT = sbA.tile([128, QC, 128], BF16, tag="tnT")
            for c in range(QC):
                tp = tpsA.tile([128, 128], BF16, tag="tpA")
                nc.tensor.transpose(tp, tn[:, c * 128:(c + 1) * 128], ident)
                nc.vector.tensor_copy(tnT[:, c, :], tp)
            kvT = sbA.tile([128, 4, 128], BF16, tag="kvT")
            for c in range(4):
                tp = tpsA.tile([128, 128], BF16, tag="tpA")
                nc.tensor.transpose(tp, kvb[:, c * 128:(c + 1) * 128], ident)
                nc.vector.tensor_copy(kvT[:, c, :], tp)
            nc.sync.dma_start(
                out=cc1i[0:TN1].rearrange("(c p s) -> p c s", p=128, s=SBLK),
                in_=tnT)
            nc.sync.dma_start(
                out=cc1i[TN1:TN1 + TK1].rearrange("(c p s) -> p c s", p=128, s=SBLK),
                in_=kvT)
            nc.sync.dma_start(
                out=cc1i[TN1 + TK1:CC1N].rearrange("(p d) -> p d", p=128),
                in_=kvb)
            nc.gpsimd.collective_compute(
                "AllGather", mybir.AluOpType.bypass, replica_groups=[CORE_IDS],
                ins=[cc1i[:]], outs=[cc1o[:]])

        cc1v = cc1o.rearrange("(g n) -> g n", g=NC)

        # ============ PHASE B: q up-projection + per-head norm/rope ========
        qallp = octx.enter_context(tc.tile_pool(name="qallp", bufs=1))
        qall = qallp.tile([128, 8, HPC * D], BF16)

        with tc.tile_pool(name="BT", bufs=1) as BT, \
             tc.tile_pool(name="wqbp", bufs=1) as wqbp, \
             tc.tile_pool(name="psB", bufs=4, space="PSUM") as psB, \
             tc.tile_pool(name="sbB", bufs=4) as sbB:
            tnT_f = BT.tile([128, QC, S], BF16)
            for c in range(QC):
                nc.sync.dma_start(
                    out=tnT_f[:, c, :].rearrange("p (g s) -> p g s", g=NC),
                    in_=cc1v[:, c * 16384:(c + 1) * 16384]
                        .rearrange("g (p s) -> p g s", p=128))
            csF_sb = BT.tile([128, 8, ROPE], F32)
            nc.sync.dma_start(out=csF_sb,
                              in_=csF_d.rearrange("(t p) c -> p t c", p=128))
            for wh in range(2):
              wqb = wqbp.tile([128, QC, 2048], BF16, tag="wqbh",
                              name=f"wqb_{wh}")
              nc.sync.dma_start(
                  out=wqb,
                  in_=wqbT_d.rearrange("(c k) n -> k c n", k=128)
                      [:, :, wh * 2048:(wh + 1) * 2048])
              for st in range(8):
                qpt = [psB.tile([128, 1024], F32, tag="qps",
                                name=f"qps_{wh}_{st}_{i}") for i in range(2)]
                for c in range(QC):
                    lh = tnT_f[:, c, st * 128:(st + 1) * 128]
                    for hl in range(4):
                        nc.tensor.matmul(
                            qpt[hl // 2][:, (hl % 2) * 512:(hl % 2 + 1) * 512],
                            lh, wqb[:, c, hl * 512:(hl + 1) * 512],
                            start=(c == 0), stop=(c == QC - 1))
                for hl in range(4):
                    if True:
                        h = wh * 4 + hl
                        qsl = qpt[hl // 2][:, (hl % 2) * 512:(hl % 2 + 1) * 512]
                        scr = sbB.tile([128, D], BF16, tag="scrB")
                        ssq = sbB.tile([128, 1], F32, tag="ssqB")
                        nc.scalar.activation(out=scr, in_=qsl, func=AF.Square,
                                             accum_out=ssq)
                        sd = sbB.tile([128, 1], F32, tag="sdB")
                        nc.scalar.activation(out=sd, in_=ssq, func=AF.Sqrt,
                                             bias=epsv, scale=1.0 / D)
                        rq = sbB.tile([128, 1], F32, tag="rqB")
                        nc.vector.reciprocal(rq, sd)
                        r2 = sbB.tile([128, 1], F32, tag="r2B")
                        nc.vector.tensor_scalar_mul(r2, rq, SCALE)
                        qf = sbB.tile([128, D], F32, tag="qfB")
                        nc.vector.tensor_scalar_mul(qf, qsl, r2)
                        cs = csF_sb[:, st, :]
                        t1 = sbB.tile([128, 32], F32, tag="t1B")
                        t2 = sbB.tile([128, 32], F32, tag="t2B")
                        t3 = sbB.tile([128, 32], F32, tag="t3B")
                        t4 = sbB.tile([128, 32], F32, tag="t4B")
                        nc.vector.tensor_mul(t1, qf[:, 448:480], cs[:, 0:32])
                        nc.vector.tensor_mul(t2, qf[:, 480:512], cs[:, 32:64])
                        nc.vector.tensor_mul(t3, qf[:, 448:480], cs[:, 32:64])
                        nc.vector.tensor_mul(t4, qf[:, 480:512], cs[:, 0:32])
                        base = h * 512
                        nc.vector.tensor_copy(qall[:, st, base:base + 448],
                                              qf[:, 0:448])
                        nc.vector.tensor_sub(qall[:, st, base + 448:base + 480],
                                             t1, t2)
                        nc.vector.tensor_add(qall[:, st, base + 480:base + 512],
                                             t3, t4)

        # ============ PHASE C/D: attention + group output projection =======
        with tc.tile_pool(name="kvp", bufs=1) as kvp, \
             tc.tile_pool(name="woap", bufs=2) as woap, \
             tc.tile_pool(name="oTp", bufs=1) as oTp, \
             tc.tile_pool(name="psS", bufs=2, space="PSUM") as psS, \
             tc.tile_pool(name="psO", bufs=2, space="PSUM") as psO, \
             tc.tile_pool(name="psT", bufs=2, space="PSUM") as psT, \
             tc.tile_pool(name="psD", bufs=2, space="PSUM") as psD, \
             tc.tile_pool(name="sbC", bufs=6) as sbC, \
             tc.tile_pool(name="pP", bufs=4) as pP, \
             tc.tile_pool(name="pTp", bufs=4) as pTp, \
             tc.tile_pool(name="sbD", bufs=4) as sbD:
            kvT_f = kvp.tile([128, 4, S], BF16)
            for c in range(4):
                nc.sync.dma_start(
                    out=kvT_f[:, c, :].rearrange("p (g s) -> p g s", g=NC),
                    in_=cc1v[:, TN1 + c * 16384:TN1 + (c + 1) * 16384]
                        .rearrange("g (p s) -> p g s", p=128))
            kv_f = kvp.tile([128, NC, D], BF16)
            for j in range(NC):
                nc.sync.dma_start(
                    out=kv_f[:, j, :],
                    in_=cc1v[j, TN1 + TK1:CC1N].rearrange("(p d) -> p d", p=128))
            oT_all = oTp.tile([128, 32, 4, 128], BF16)
            for st in range(8):
                for h in range(HPC):
                    qt_ps = psT.tile([128, 4, 128], BF16, tag="tps")
                    for dc in range(4):
                        nc.tensor.transpose(
                            qt_ps[:, dc, :],
                            qall[:, st, h * 512 + dc * 128:h * 512 + (dc + 1) * 128],
                            ident)
                    qt = sbC.tile([128, 4, 128], BF16, tag="qt")
                    nc.vector.tensor_copy(qt, qt_ps)
                    ncols = (st + 1) * 128
                    chunks = [(0, min(512, ncols))]
                    if ncols > 512:
                        chunks.append((512, ncols - 512))
                    scps, mxs = [], []
                    for (off, w) in chunks:
                        sp = psS.tile([128, 512], F32, tag="sc")
                        for dc in range(4):
                            nc.tensor.matmul(sp[:, :w], qt[:, dc, :],
                                             kvT_f[:, dc, off:off + w],
                                             start=(dc == 0), stop=(dc == 3))
                        if off + w == ncols:
                            nc.vector.tensor_add(sp[:, w - 128:w],
                                                 sp[:, w - 128:w], cmask_sb)
                        mx = sbC.tile([128, 1], F32, tag="mx")
                        nc.vector.reduce_max(out=mx, in_=sp[:, :w], axis=AX.X)
                        scps.append((sp, off, w))
                        mxs.append(mx)
                    mm = sbC.tile([128, 1], F32, tag="mm")
                    if len(mxs) == 2:
                        nc.vector.tensor_max(mm, mxs[0], mxs[1])
                    else:
                        nc.vector.tensor_copy(mm, mxs[0])
                    m2 = sbC.tile([128, 1], F32, tag="m2")
                    nc.vector.tensor_max(m2, mm, sinkb_sb[:, h:h + 1])
                    negm = sbC.tile([128, 1], F32, tag="negm")
                    nc.vector.tensor_scalar_mul(negm, m2, -1.0)
                    psb = pP.tile([128, 1024], BF16, tag="p")
                    sums = []
                    for (sp, off, w) in scps:
                        sm = sbC.tile([128, 1], F32, tag="sm")
                        nc.scalar.activation(out=psb[:, off:off + w],
                                             in_=sp[:, :w], func=AF.Exp,
                                             bias=negm, scale=1.0, accum_out=sm)
                        sums.append(sm)
                    se = sbC.tile([128, 1], F32, tag="se")
                    nc.scalar.activation(out=se, in_=sinkb_sb[:, h:h + 1],
                                         func=AF.Exp, bias=negm, scale=1.0)
                    dn = sbC.tile([128, 1], F32, tag="dn")
                    nc.vector.tensor_add(dn, sums[0], se)
                    if len(sums) == 2:
                        dn2 = sbC.tile([128, 1], F32, tag="dn2")
                        nc.vector.tensor_add(dn2, dn, sums[1])
                        dn = dn2
                    rd = sbC.tile([128, 1], F32, tag="rd")
                    nc.vector.reciprocal(rd, dn)
                    pT = pTp.tile([128, 8, 128], BF16, tag="pT")
                    for j in range(st + 1):
                        nc.sync.dma_start(out=pT[:, j, :],
                                          in_=psb[:, j * 128:(j + 1) * 128],
                                          transpose=True)
                    ops = psO.tile([128, 512], F32, tag="o")
                    for j in range(st + 1):
                        nc.tensor.matmul(ops, pT[:, j, :], kv_f[:, j, :],
                                         start=(j == 0), stop=(j == st))
                    obf = sbC.tile([128, D], BF16, tag="obf")
                    nc.vector.tensor_scalar_mul(obf, ops, rd)
                    ot_ps = psT.tile([128, 4, 128], BF16, tag="tps")
                    for dc in range(4):
                        nc.tensor.transpose(ot_ps[:, dc, :],
                                            obf[:, dc * 128:(dc + 1) * 128],
                                            ident)
                    nc.vector.tensor_copy(oT_all[:, h * 4:(h + 1) * 4, st % 4, :],
                                          ot_ps)
                if st in (3, 7):
                    half = st // 4
                    for rc in range(8):
                        woa_rc = woap.tile([128, 32, 128], BF16, tag="woa",
                                           name=f"woa_{half}_{rc}")
                        nc.sync.dma_start(
                            out=woa_rc,
                            in_=woaT_d.rearrange("(c k) n -> k c n", k=128)
                                [:, :, rc * 128:(rc + 1) * 128])
                        dps = psD.tile([128, 512], F32, tag="dps")
                        for dc in range(32):
                            nc.tensor.matmul(
                                dps, woa_rc[:, dc, :],
                                oT_all[:, dc, :, :].rearrange("p a b -> p (a b)"),
                                start=(dc == 0), stop=(dc == 31))
                        ob = sbD.tile([128, 512], BF16, tag="ob")
                        nc.vector.tensor_copy(ob, dps)
                        nc.sync.dma_start(
                            out=cc2i[half][rc * 65536:(rc + 1) * 65536]
                                .rearrange("(p s) -> p s", p=128),
                            in_=ob)
                    nc.gpsimd.collective_compute(
                        "AllGather", mybir.AluOpType.bypass,
                        replica_groups=[CORE_IDS],
                        ins=[cc2i[half][:]], outs=[cc2o[half][:]])

        # ================= PHASE E: final dense projection =================
        with tc.tile_pool(name="wobp", bufs=1) as wobp, \
             tc.tile_pool(name="psE", bufs=2, space="PSUM") as psE, \
             tc.tile_pool(name="ctp", bufs=4) as ctp, \
             tc.tile_pool(name="sbE", bufs=3) as sbE:
            wob = wobp.tile([128, 64, OUTC], BF16)
            nc.sync.dma_start(out=wob,
                              in_=wobT_d.rearrange("(c k) n -> k c n", k=128))
            for sb2 in range(2):
                cc2v = cc2o[sb2].rearrange("(g n) -> g n", g=NC)
                for sp2 in range(2):
                    eps_t = [psE.tile([128, OUTC], F32, tag="eps",
                                      name=f"eps_{sb2}_{sp2}_{i}")
                             for i in range(2)]
                    for rc in range(64):
                        gp, j = rc // 8, rc % 8
                        ct = ctp.tile([128, 512], BF16, tag="ct")
                        nc.sync.dma_start(
                            out=ct,
                            in_=cc2v[gp, j * 65536:(j + 1) * 65536]
                                .rearrange("(p s) -> p s", p=128))
                        for e2 in range(2):
                            stl = sp2 * 2 + e2
                            lh = ct[:, stl * 128:(stl + 1) * 128]
                            nc.tensor.matmul(eps_t[e2][:, 0:512], lh,
                                             wob[:, rc, 0:512],
                                             start=(rc == 0), stop=(rc == 63))
                            nc.tensor.matmul(eps_t[e2][:, 512:OUTC], lh,
                                             wob[:, rc, 512:OUTC],
                                             start=(rc == 0), stop=(rc == 63))
                    for e2 in range(2):
                        stile = sb2 * 4 + sp2 * 2 + e2
                        of = sbE.tile([128, OUTC], F32, tag="of")
                        nc.vector.tensor_copy(of, eps_t[e2])
                        nc.sync.dma_start(
                            out=outY_d[stile * 128:(stile + 1) * 128, :],
                            in_=of)

    nc.compile()
    return nc


def _host_prep(x, freqs_cis, wq_a, q_norm_w, wq_b, wkv, kv_norm_w,
               wo_a_w, wo_b, attn_sink):
    perm = np.concatenate([np.arange(NOPE),
                           NOPE + 2 * np.arange(ROPE // 2),
                           NOPE + 1 + 2 * np.arange(ROPE // 2)])
    x2 = np.asarray(x, np.float32).reshape(S, HID)
    wqa_T = np.asarray(wq_a, np.float32).T                      # [HID, QL]
    wkv_p = np.asarray(wkv, np.float32)[perm, :]                # [D, HID]
    wA = np.ascontiguousarray(
        np.concatenate([wqa_T, wkv_p.T], axis=1)).astype(BF)    # [HID, 2048]
    wqb_eff = np.asarray(wq_b, np.float32) * np.asarray(q_norm_w, np.float32)[None, :]
    wqb_r = wqb_eff.reshape(H, D, QL)[:, perm, :]               # [H, D, QL]
    fc = np.asarray(freqs_cis, np.float32)
    csF = np.ascontiguousarray(
        np.concatenate([fc[:, :, 0], fc[:, :, 1]], axis=1))     # [S, 64]
    kvw = np.asarray(kv_norm_w, np.float32)[perm]
    kvwb = np.ascontiguousarray(np.tile(kvw[None, :], (128, 1)))
    woa = np.asarray(wo_a_w, np.float32).reshape(G, R, HPG, D)[:, :, :, perm] \
        .reshape(G, R, HPG * D)
    wob = np.asarray(wo_b, np.float32)
    ii = np.arange(128)
    cmask = np.where(ii[None, :] <= ii[:, None], 0.0, -1e30).astype(np.float32)
    sink = np.asarray(attn_sink, np.float32)

    in_maps = []
    for g in range(NC):
        xT = np.ascontiguousarray(x2[g * SBLK:(g + 1) * SBLK, :].T).astype(BF)
        wqbT = np.ascontiguousarray(
            wqb_r[g * HPC:(g + 1) * HPC].reshape(HPC * D, QL).T).astype(BF)
        woaT = np.ascontiguousarray(woa[g].T).astype(BF)
        wobT = np.ascontiguousarray(
            wob[g * OUTC:(g + 1) * OUTC, :].T).astype(BF)
        sinkb = np.ascontiguousarray(
            np.tile(sink[g * HPC:(g + 1) * HPC][None, :], (128, 1)))
        csA = np.ascontiguousarray(csF[g * SBLK:(g + 1) * SBLK])
        in_maps.append({
            "xT": xT, "wA": wA, "wqbT": wqbT, "woaT": woaT, "wobT": wobT,
            "kvwb": kvwb, "csA": csA, "csF": csF, "sinkb": sinkb,
            "cmask": cmask,
        })
    return in_maps


def _make_runner(nc, chain=1, donate=True):
    """Build the jitted 8-core PJRT executor once (mirrors the multi-core
    branch of bass2jax.run_bass_via_pjrt, but caches the jitted callable).

    chain>1 executes the NEFF `chain` times back-to-back with a data
    dependency through the donated output buffer — used to measure device
    execution time as a slope, independent of host/tunnel transfer costs."""
    import jax
    from jax.experimental.shard_map import shard_map
    from jax.sharding import Mesh, PartitionSpec
    from concourse import bass2jax

    bass2jax.install_neuronx_cc_hook()
    partition_name = (nc.partition_id_tensor.name
                      if nc.partition_id_tensor else None)
    in_names, out_names, out_avals = [], [], []
    for alloc in nc.m.functions[0].allocations:
        if not isinstance(alloc, mybir.MemoryLocationSet):
            continue
        name = alloc.memorylocations[0].name
        if alloc.kind == "ExternalInput":
            if name != partition_name:
                in_names.append(name)
        elif alloc.kind == "ExternalOutput":
            out_names.append(name)
            out_avals.append(jax.core.ShapedArray(
                tuple(alloc.tensor_shape), mybir.dt.np(alloc.dtype)))
    n_params = len(in_names)
    all_names = list(in_names) + list(out_names)
    if partition_name is not None:
        all_names.append(partition_name)
    all_names = tuple(all_names)
    donate_idx = (tuple(range(n_params, n_params + len(out_names)))
                  if donate else ())

    def _body(*args):
        ins = list(args[:n_params])
        outs = list(args[n_params:])
        for _ in range(chain):
            operands = ins + outs
            if partition_name is not None:
                operands.append(bass2jax.partition_id_tensor())
            outs = list(bass2jax._bass_exec_p.bind(
                *operands, out_avals=tuple(out_avals), in_names=all_names,
                out_names=tuple(out_names), lowering_input_output_aliases=(),
                sim_require_finite=True, sim_require_nnan=True, nc=nc))
        return tuple(outs)

    devices = jax.devices()[:NC]
    mesh = Mesh(np.asarray(devices), ("core",))
    in_specs = (PartitionSpec("core"),) * (n_params + len(out_names))
    out_specs = (PartitionSpec("core"),) * len(out_names)
    sharded = jax.jit(
        shard_map(_body, mesh=mesh, in_specs=in_specs, out_specs=out_specs,
                  check_rep=False),
        donate_argnums=donate_idx, keep_unused=True)
    return {"sharded": sharded, "in_names": in_names, "out_names": out_names,
            "out_avals": out_avals, "mesh": mesh}


def get_runner(chain=1, donate=True):
    key = f"runner_{chain}_{donate}"
    if key not in _CACHE:
        if "nc" not in _CACHE:
            _CACHE["nc"] = _build()
        _CACHE[key] = _make_runner(_CACHE["nc"], chain=chain, donate=donate)
    return _CACHE[key]


def concat_inputs(in_maps, runner):
    return [np.concatenate([in_maps[c][n] for c in range(NC)], axis=0)
            for n in runner["in_names"]]


def make_zero_outs(runner):
    return [np.zeros((NC * av.shape[0], *av.shape[1:]), av.dtype)
            for av in runner["out_avals"]]


def kernel(x, freqs_cis, wq_a, q_norm_w, wq_b, wkv, kv_norm_w,
           wo_a_w, wo_b, attn_sink):
    in_maps = _host_prep(x, freqs_cis, wq_a, q_norm_w, wq_b, wkv, kv_norm_w,
                         wo_a_w, wo_b, attn_sink)
    try:
        runner = get_runner(donate=False)
        out_arrs = runner["sharded"](*concat_inputs(in_maps, runner),
                                     *make_zero_outs(runner))
        idx = runner["out_names"].index("outY")
        outY = np.asarray(out_arrs[idx]).reshape(NC, S, OUTC)
    except Exception:
        # fall back to the stock SPMD runner (same axon/PJRT path, uncached)
        if "nc" not in _CACHE:
            _CACHE["nc"] = _build()
        res = run_bass_kernel_spmd(_CACHE["nc"], in_maps, CORE_IDS)
        outY = np.stack([res.results[g]["outY"] for g in range(NC)])
    out = np.empty((1, S, HID), np.float32)
    for g in range(NC):
        out[0, :, g * OUTC:(g + 1) * OUTC] = outY[g]
    return out
